# revision 1
# baseline (speedup 1.0000x reference)
"""Trainium2 Bass kernel for nn_AttentionLayer (B=4, T=2048, C=1024, H=16).

Sharding (8 cores): core c = (batch b = c//2, head-group g = c%2).
Data parallel on batch, tensor parallel on heads: each core computes the
qkv projection for its 8 heads, causal flash-attention, and a partial
output projection (row split of w_proj). Host sums the two partials per
batch and re-transposes.

Per-core kernel (Bass/Tile, fp32r matmuls = TF32-like fp22 PE mode):
  phase A: qkv projection.  Q^T/K^T produced in [head_dim, t] layout
           (moving operand = x^T), V in natural [t, head_dim] layout
           (moving operand = w_v^T) with an appended ones column.
  phase B: causal attention per head-pair.  S^T = K^T.T @ Q^T row-tiled
           2 heads/matmul (contraction 64 x 2), exp on ACT (no
           max-subtract needed: logits are O(1)), causal mask by DVE
           multiply with constant triangle tiles (diagonal tiles are
           width-narrowed), O^T = [V|1].T @ P^T accumulated in PSUM;
           row 64 gives softmax denominators; normalize via DVE
           reciprocal + K=1 selector matmul (partition broadcast on PE).
  phase C: out^T = w_p^T.T @ y^T + bias (bias only on g=0 cores).
  Phases are software-pipelined: emission interleaves projection chunks
  with pending attention/output chunks so the in-order PE stream always
  has matmul filler during exp stalls.

All DRAM tensors are host-pre-tiled so every DMA is one contiguous block.
"""
from contextlib import ExitStack

import numpy as np

import concourse.bacc as bacc
import concourse.mybir as mybir
import concourse.tile as tile
from concourse.bass_utils import run_bass_kernel_spmd

F32 = mybir.dt.float32
F32R = mybir.dt.float32r
AF = mybir.ActivationFunctionType

B, T, C, H = 4, 2048, 1024, 16
HD = C // H          # 64
NH = H // 2          # heads per core: 8
QCOLS = NH * HD      # 512


def build(T=T, C=C, NH=NH, HD=HD, TQ=512, loop_iters=1):
    assert C % 128 == 0 and T % TQ == 0 and TQ % 128 == 0
    NP = NH // 2              # head pairs
    CT = C // 128             # contraction tiles
    NTB = T // TQ             # time blocks
    TT = T // 128             # tk tiles
    NO = C // 128             # out row tiles
    QC = NH * HD
    scale = 1.0 / (HD ** 0.5)

    nc = bacc.Bacc()
    xT = nc.declare_dram_parameter("xT", [CT, NTB, 128, TQ], F32R, isOutput=False)
    wqkT = nc.declare_dram_parameter("wqkT", [2, CT, 128, QC], F32R, isOutput=False)
    wvT = nc.declare_dram_parameter("wvT", [CT, 128, QC], F32R, isOutput=False)
    wpT = nc.declare_dram_parameter("wpT", [NP, 128, C], F32R, isOutput=False)
    bias = nc.declare_dram_parameter("bias", [128, NO], F32, isOutput=False)
    outT = nc.declare_dram_parameter("outT", [NO, NTB, 128, TQ], F32, isOutput=True)

    with tile.TileContext(nc) as tc, ExitStack() as ctx:
        # long-lived pools first (stack allocator)
        qt_pool = ctx.enter_context(tc.tile_pool(name="qt", bufs=NP * NTB))
        kt_pool = ctx.enter_context(tc.tile_pool(name="kt", bufs=NP * NTB))
        v_pool = ctx.enter_context(tc.tile_pool(name="v", bufs=TT))
        wp_pool = ctx.enter_context(tc.tile_pool(name="wp", bufs=NP))
        bias_pool = ctx.enter_context(tc.tile_pool(name="bias", bufs=1))

        bias_sb = bias_pool.tile([128, NO], F32, tag="bias", name="bias_sb")
        nc.sync.dma_start(bias_sb[:], bias[:])
        ones_sb = bias_pool.tile([128, NH], F32, tag="ones", name="ones_sb")
        nc.gpsimd.memset(ones_sb[:], 1.0)
        # causal band masks: mask_a[x,y]=1 iff y>=x (used for delta<TQ-256);
        # mask_b[x,y]=1 iff y>=x+128 (used for the clamped delta=TQ-256 tile)
        ii = np.arange(128)[:, None]
        mask_a_np = (np.arange(128)[None, :] >= ii).astype(np.float32)
        mask_b_np = (np.arange(256)[None, :] >= ii + 128).astype(np.float32)
        sel_np = np.ones((1, 64), np.float32)
        mask_a_dram = nc.inline_tensor(mask_a_np, name="mask_a")
        mask_b_dram = nc.inline_tensor(mask_b_np, name="mask_b")
        sel_dram = nc.inline_tensor(sel_np, name="sel")
        mask_a = bias_pool.tile([128, 128], F32R, tag="mask_a", name="mask_a_sb")
        mask_b = bias_pool.tile([128, 256], F32R, tag="mask_b", name="mask_b_sb")
        sel_sb = bias_pool.tile([1, 64], F32R, tag="sel", name="sel_sb")
        nc.gpsimd.dma_start(mask_a[:], mask_a_dram[:])
        nc.gpsimd.dma_start(mask_b[:], mask_b_dram[:])
        nc.gpsimd.dma_start(sel_sb[:], sel_dram[:])
        wp_sb = [wp_pool.tile([128, C], F32R, tag="wp", name="wp") for p in range(NP)]
        for p in range(NP):
            nc.sync.dma_start(wp_sb[p][:], wpT[p])

        qt = {}
        kt = {}
        vt = []
        yt = {}

        def body():
            qt.clear(); kt.clear(); vt.clear(); yt.clear()
            ctx2 = ExitStack()
            st_pool = ctx2.enter_context(tc.tile_pool(name="st", bufs=2, space="PSUM"))
            o_ps_pool = ctx2.enter_context(tc.tile_pool(name="ops", bufs=4, space="PSUM"))
            pt_pool = ctx2.enter_context(tc.tile_pool(name="pt", bufs=4))
            rc_pool = ctx2.enter_context(tc.tile_pool(name="rc", bufs=4))
            osb_pool = ctx2.enter_context(tc.tile_pool(name="osb", bufs=3))

            def gen_proj_block(tb, wqk_pool, xs_pool, wvs):
                """Phase A chunk generator: yields after each matmul group."""
                xs = [xs_pool.tile([128, TQ], F32R, tag="xs", name="xs") for _ in range(CT)]
                for c in range(CT):
                    nc.sync.dma_start(xs[c][:], xT[c, tb])
                # half 0 = Q cols, half 1 = K cols of wqkT
                for half in range(2):
                    ws = []
                    for c in range(CT):
                        w = wqk_pool.tile([128, QC], F32R, tag="wqk", name="wqk")
                        nc.sync.dma_start(w[:], wqkT[half, c])
                        ws.append(w)
                    for jp in range(NP):
                        jt = half * NP + jp
                        ps = o_ps_pool.tile([128, TQ], F32, tag="ops", name="mm")
                        for c in range(CT):
                            nc.tensor.matmul(ps[:], ws[c][:, 128 * jp:128 * (jp + 1)], xs[c][:],
                                             start=(c == 0), stop=(c == CT - 1))
                        dst = qt_pool.tile([128, TQ], F32R, tag="qt", name="qt") if jt < NP else kt_pool.tile([128, TQ], F32R, tag="kt", name="kt")
                        nc.vector.tensor_copy(dst[:], ps[:])
                        if jt < NP:
                            qt[(jt, tb)] = dst
                        else:
                            kt[(jt - NP, tb)] = dst
                        yield
                for ti in range(TQ // 128):
                    tt_i = tb * (TQ // 128) + ti
                    ps = o_ps_pool.tile([128, QC], F32, tag="ops", name="mmv")
                    for c in range(CT):
                        nc.tensor.matmul(ps[:], xs[c][:, 128 * ti:128 * (ti + 1)], wvs[c][:],
                                         start=(c == 0), stop=(c == CT - 1))
                    vtile = v_pool.tile([128, NH * (HD + 1)], F32R, tag="v", name="v")
                    v3 = vtile[:].rearrange("p (h d) -> p h d", d=HD + 1)
                    nc.vector.tensor_copy(v3[:, :, 0:HD], ps[:].rearrange("p (h d) -> p h d", d=HD))
                    nc.vector.tensor_copy(v3[:, :, HD], ones_sb[:])
                    assert len(vt) == tt_i
                    vt.append(vtile)
                    yield

            def gen_attention_block(qi):
                """Phase B generator (all pairs, one query block) + phase C."""
                tq0 = qi * TQ
                ntk = (tq0 + TQ) // 128
                def emit_norm(state):
                    p_, o0_, o1_ = state
                    ytile = qt_pool.tile([128, TQ], F32R, tag="qt", name="y")
                    yt[(p_, qi)] = ytile
                    rcA = rc_pool.tile([1, TQ], F32R, tag="rc", name="rcA")
                    rcB = rc_pool.tile([1, TQ], F32R, tag="rcb", name="rcB")
                    with nc.allow_low_precision(reason="f32r==fp32 bits; denominators kept full fp32"):
                        nc.vector.reciprocal(rcA[:], o0_[HD:HD + 1, :])
                        nc.vector.reciprocal(rcB[:], o1_[HD:HD + 1, :])
                    bc0 = o_ps_pool.tile([HD, TQ], F32, tag="ops", name="bc0")
                    bc1 = o_ps_pool.tile([HD, TQ], F32, tag="ops", name="bc1")
                    nc.tensor.matmul(bc0[:], sel_sb[:], rcA[:], start=True, stop=True)
                    nc.tensor.matmul(bc1[:], sel_sb[:], rcB[:], start=True, stop=True)
                    nc.scalar.activation(ytile[0:64, :], o0_[0:HD, :], AF.Copy)
                    nc.scalar.activation(ytile[64:128, :], o1_[0:HD, :], AF.Copy)
                    nc.vector.tensor_mul(ytile[0:64, :], ytile[0:64, :], bc0[:])
                    nc.vector.tensor_mul(ytile[64:128, :], ytile[64:128, :], bc1[:])

                pending_norm = None
                for p in range(NP):
                    h0 = 2 * p
                    h1 = 2 * p + 1
                    o0 = o_ps_pool.tile([HD + 1, TQ], F32, tag="ops", name="ops")
                    o1 = o_ps_pool.tile([HD + 1, TQ], F32, tag="ops", name="ops2")

                    def emit_av(state):
                        pt_, w_, dlt_, tki_ = state
                        vtile = vt[tki_]
                        v3 = vtile[:].rearrange("p (h d) -> p h d", d=HD + 1)
                        nc.tensor.matmul(o0[:, dlt_:TQ], v3[:, h0, :], pt_[:, 0:w_],
                                         start=(tki_ == 0), stop=(tki_ == ntk - 1))
                        nc.tensor.matmul(o1[:, dlt_:TQ], v3[:, h1, :], pt_[:, w_:2 * w_],
                                         start=(tki_ == 0), stop=(tki_ == ntk - 1))

                    pending_av = None
                    for tki in range(ntk):
                        tk0 = tki * 128
                        # diagonal narrowing: only q >= tk0 attends; min width
                        # 256 (f32r matmul drops to 1/4 rate below 256)
                        dlt = min(max(0, tk0 - tq0), TQ - 256)
                        w = TQ - dlt
                        diag = tk0 >= tq0
                        ktile = kt[(p, tk0 // TQ)]
                        koff = tk0 % TQ
                        qtile = qt[(p, qi)]
                        st = st_pool.tile([128, 2 * TQ], F32, tag="st", name="st")
                        nc.tensor.matmul(st[:, 0:w], ktile[0:64, koff:koff + 128],
                                         qtile[0:64, dlt:TQ], start=True, stop=True)
                        nc.tensor.matmul(st[:, TQ:TQ + w], ktile[64:128, koff:koff + 128],
                                         qtile[64:128, dlt:TQ], start=True, stop=True)
                        pt = pt_pool.tile([128, 2 * TQ], F32R, tag="pt", name="pt")
                        st_v = st[:].rearrange("p (h q) -> p h q", q=TQ)[:, :, 0:w]
                        pt_v = pt[:, 0:2 * w].rearrange("p (h q) -> p h q", h=2)
                        nc.scalar.activation(pt_v, st_v, AF.Exp, scale=scale)
                        if diag:
                            clamped = (tk0 - tq0) > dlt
                            m = mask_b if clamped else mask_a
                            bw = 256 if clamped else 128
                            band = pt[:, 0:2 * w].rearrange("p (h q) -> p h q", h=2)[:, :, 0:bw]
                            nc.vector.tensor_mul(band, band,
                                                 m[:, 0:bw].rearrange("p q -> p () q").broadcast_to((128, 2, bw)))
                        # one-deep rotation: AV(i-1) is emitted after S(i) so
                        # the PE always has the next S-pair during exp stalls;
                        # the previous segment's normalize is likewise deferred
                        # past this segment's first S/exp
                        if tki == 0 and pending_norm is not None:
                            emit_norm(pending_norm)
                            pending_norm = None
                            yield
                        if pending_av is not None:
                            emit_av(pending_av)
                            yield
                        pending_av = (pt, w, dlt, tki)
                    emit_av(pending_av)
                    pending_norm = (p, o0, o1)
                    yield
                    yield
                if pending_norm is not None:
                    emit_norm(pending_norm)
                    pending_norm = None
                    yield
                # phase C for this time block
                tb = qi
                for ot in range(NO):
                    ps = o_ps_pool.tile([128, TQ], F32, tag="ops", name="mmo")
                    for p in range(NP):
                        nc.tensor.matmul(ps[:], wp_sb[p][:, 128 * ot:128 * (ot + 1)], yt[(p, tb)][:],
                                         start=(p == 0), stop=(p == NP - 1))
                    osb = osb_pool.tile([128, TQ], F32, tag="osb", name="osb")
                    nc.vector.tensor_scalar_add(osb[:], ps[:], bias_sb[:, ot:ot + 1])
                    nc.sync.dma_start(outT[ot, tb], osb[:])
                    yield

            def drain(gen):
                if gen is None:
                    return None
                try:
                    next(gen)
                    return gen
                except StopIteration:
                    return None

            # software pipeline: interleave phase-A chunks of block tb with
            # pending phase-B/C chunks so the PE instruction stream always
            # has projection matmuls to fill attention stalls.
            from collections import deque
            with tc.tile_pool(name="wv_s", bufs=CT) as wv_pool, \
                 tc.tile_pool(name="wqk_s", bufs=CT + 2) as wqk_pool, \
                 tc.tile_pool(name="xs", bufs=CT + 1) as xs_pool:
                wvs = []
                for c in range(CT):
                    w = wv_pool.tile([128, QC], F32R, tag="wv", name="wv")
                    nc.sync.dma_start(w[:], wvT[c])
                    wvs.append(w)
                pending = deque()

                def step_att():
                    while pending:
                        try:
                            next(pending[0])
                            return True
                        except StopIteration:
                            pending.popleft()
                    return False

                for tb in range(NTB):
                    for _ in gen_proj_block(tb, wqk_pool, xs_pool, wvs):
                        step_att()
                    pending.append(gen_attention_block(tb))
                while pending:
                    step_att()
            ctx2.close()

        if loop_iters == 1:
            body()
        else:
            with tc.For_i(0, loop_iters, 1):
                body()
    nc.finalize()
    return nc


def _tile2d(a, pr, pc):
    """[R, S] -> [R//pr, S//pc, pr, pc] contiguous tiles."""
    R, S = a.shape
    return np.ascontiguousarray(
        a.reshape(R // pr, pr, S // pc, pc).transpose(0, 2, 1, 3))


def shard_inputs(x, w_attn, w_proj, b_proj, TQ=512):
    """Returns in_maps for 8 cores: core c = (b=c//2, g=c%2)."""
    CT = C // 128
    NP = NH // 2
    NTB = T // TQ
    wq, wk, wv = w_attn[0:C], w_attn[C:2 * C], w_attn[2 * C:3 * C]
    x = np.asarray(x)
    in_maps = []
    for core in range(8):
        b = core // 2
        g = core % 2
        rows = slice(g * QCOLS, (g + 1) * QCOLS)
        xTt = _tile2d(np.asarray(x[b]).T, 128, TQ)                       # [CT,NTB,128,TQ]
        wqkT_flat = np.concatenate([wq[rows], wk[rows]], 0).T      # [C, 2QC]
        wqkTt = np.ascontiguousarray(
            wqkT_flat.reshape(CT, 128, 2, QCOLS).transpose(2, 0, 1, 3))  # [2,CT,128,QC]
        wvTt = np.ascontiguousarray(wv[rows].T.reshape(CT, 128, QCOLS))
        wpTt = np.ascontiguousarray(w_proj[:, rows].T.reshape(NP, 128, C))
        in_maps.append({
            "xT": xTt,
            "wqkT": wqkTt,
            "wvT": wvTt,
            "wpT": wpTt,
            "bias": (np.ascontiguousarray(b_proj.reshape(C // 128, 128).T)
                     if g == 0 else np.zeros((128, C // 128), np.float32)),
        })
    return in_maps


def unshard_output(outT_tiles_pair, TQ=512):
    """outT [NO,NTB,128,TQ] partials (2 cores) -> out [T, C]."""
    s = outT_tiles_pair[0] + outT_tiles_pair[1]
    NO, NTB = C // 128, T // TQ
    return s.transpose(0, 2, 1, 3).reshape(C, T).T


_NC_CACHE = {}


def kernel(x, w_attn, w_proj, b_proj):
    if "nc" not in _NC_CACHE:
        _NC_CACHE["nc"] = build()
    nc = _NC_CACHE["nc"]
    in_maps = shard_inputs(x, w_attn, w_proj, b_proj)
    res = run_bass_kernel_spmd(nc, in_maps, core_ids=list(range(8)))
    out = np.empty((B, T, C), np.float32)
    for b in range(B):
        out[b] = unshard_output([res.results[2 * b]["outT"],
                                 res.results[2 * b + 1]["outT"]])
    return out



# revision 7
# speedup vs baseline: 1.1199x; 1.1199x over previous
"""Trainium2 Bass kernel for nn_AttentionLayer (B=4, T=2048, C=1024, H=16).

Sharding (8 cores): core c = (batch b = c//2, head-group g = c%2).
Data parallel on batch, tensor parallel on heads: each core computes the
qkv projection for its 8 heads, causal flash-attention, and a partial
output projection (row split of w_proj). Host sums the two partials per
batch and re-transposes.

Per-core kernel (Bass/Tile):
  phase A: qkv projection in f32r (TF32-like).  Q^T/K^T produced in
           [head_dim, t] layout, V in [t, head_dim] layout with an
           appended ones column; all stored bf16 in SBUF.
  phase B: causal attention per head-pair.  S^T = K^T.T @ Q^T, the two
           heads of a pair issued to PE row-groups (0,0)/(64,0) so they
           run concurrently on HW; exp on ACT (PSUM f32 in, bf16 out);
           causal mask by DVE multiply (bf16, 2x mode) on the single
           diagonal 128-col band; O^T = [V|1].T @ P^T accumulated in
           PSUM; row 64 gives softmax denominators; normalize via DVE
           reciprocal + one K=2 selector matmul (partition broadcast)
           + DVE multiply that also does the PSUM->SBUF move.
  phase C: out^T = w_p^T.T @ y^T in bf16 + bias (bias only on g=0).

  Emission uses a virtual-clock list scheduler: attention chunks are
  emitted in dependency order, and whenever the PE stream would stall
  on ACT (exp) latency, projection / output-projection matmuls are
  spliced in as filler.  PSUM rings are dedicated (st / o / grp) so
  long-lived accumulators never alias short-lived group tiles.

All DRAM tensors are host-pre-tiled so every DMA is one contiguous block.
"""
from collections import deque
from contextlib import ExitStack

import numpy as np

import concourse.bacc as bacc
import concourse.mybir as mybir
import concourse.tile as tile
from concourse.bass_utils import run_bass_kernel_spmd

F32 = mybir.dt.float32
F32R = mybir.dt.float32r
BF16 = mybir.dt.bfloat16
AF = mybir.ActivationFunctionType

B, T, C, H = 4, 2048, 1024, 16
HD = C // H          # 64
NH = H // 2          # heads per core: 8
QCOLS = NH * HD      # 512

# virtual-clock cost model (HW-calibrated, ns)
def _mm(fd):
    return (6.0 + fd) / 2.4

def _act(fd):
    return (172.0 + fd) / 1.2

def _dve(fd, fixed=151.0, acc=1.0):
    return (fixed + fd / acc) / 0.96


def build(T=T, C=C, NH=NH, HD=HD, TQ=512, loop_iters=1):
    assert C % 128 == 0 and T % TQ == 0 and TQ % 128 == 0
    NP = NH // 2              # head pairs
    CT = C // 128             # contraction tiles
    NTB = T // TQ             # time blocks
    TT = T // 128             # tk tiles
    NO = C // 128             # out row tiles
    QC = NH * HD
    scale = 1.0 / (HD ** 0.5)

    nc = bacc.Bacc()
    xT = nc.declare_dram_parameter("xT", [CT, NTB, 128, TQ], F32R, isOutput=False)
    wqkT = nc.declare_dram_parameter("wqkT", [2, CT, 128, QC], F32R, isOutput=False)
    wvT = nc.declare_dram_parameter("wvT", [CT, 128, QC], F32R, isOutput=False)
    wpT = nc.declare_dram_parameter("wpT", [NP, 128, C], BF16, isOutput=False)
    bias = nc.declare_dram_parameter("bias", [128, NO], F32, isOutput=False)
    outT = nc.declare_dram_parameter("outT", [NO, NTB, 128, TQ], F32, isOutput=True)

    with tile.TileContext(nc) as tc, ExitStack() as ctx:
        # long-lived pools first (stack allocator)
        const_pool = ctx.enter_context(tc.tile_pool(name="const", bufs=1))
        wqk_pool = ctx.enter_context(tc.tile_pool(name="wqk", bufs=2 * CT))
        wv_pool = ctx.enter_context(tc.tile_pool(name="wv", bufs=CT))
        wp_pool = ctx.enter_context(tc.tile_pool(name="wp", bufs=NP))
        qt_pool = ctx.enter_context(tc.tile_pool(name="qt", bufs=NP * NTB))
        kt_pool = ctx.enter_context(tc.tile_pool(name="kt", bufs=NP * NTB))
        yt_pool = ctx.enter_context(tc.tile_pool(name="yt", bufs=NP * NTB))
        v_pool = ctx.enter_context(tc.tile_pool(name="v", bufs=TT))
        xs_pool = ctx.enter_context(tc.tile_pool(name="xs", bufs=2 * CT))
        pt_pool = ctx.enter_context(tc.tile_pool(name="pt", bufs=4))
        rc_pool = ctx.enter_context(tc.tile_pool(name="rc", bufs=3))
        osb_pool = ctx.enter_context(tc.tile_pool(name="osb", bufs=3))

        bias_sb = const_pool.tile([128, NO], F32, tag="bias", name="bias_sb")
        nc.sync.dma_start(bias_sb[:], bias[:])
        ones_sb = const_pool.tile([128, NH], BF16, tag="ones", name="ones_sb")
        nc.gpsimd.memset(ones_sb[:], 1.0)
        # causal band mask: mask_a[x,y]=1 iff y>=x, applied to the single
        # 128-wide diagonal band of each S tile
        bf = mybir.dt.np(BF16)
        ii = np.arange(128)[:, None]
        # negtri[k, q] = -BIG where q < k: accumulated onto diagonal S bands
        # via an identity-stationary matmul, so exp yields exact zeros there.
        negtri_np = np.where(np.arange(128)[None, :] < ii, -30000.0, 0.0).astype(bf)
        eye_np = np.eye(128, dtype=np.float32).astype(bf)
        sel_np = np.ones((1, 64), np.float32)
        negtri_dram = nc.inline_tensor(negtri_np, name="negtri")
        eye_dram = nc.inline_tensor(eye_np, name="eye")
        sel_dram = nc.inline_tensor(sel_np, name="sel")
        negtri = const_pool.tile([128, 128], BF16, tag="negtri", name="negtri_sb")
        eye_sb = const_pool.tile([128, 128], BF16, tag="eye", name="eye_sb")
        sel_sb = const_pool.tile([1, 64], F32R, tag="sel", name="sel_sb")
        nc.gpsimd.dma_start(negtri[:], negtri_dram[:])
        nc.gpsimd.dma_start(eye_sb[:], eye_dram[:])
        nc.gpsimd.dma_start(sel_sb[:], sel_dram[:])

        # persistent weights (loaded once, resident in SBUF)
        wp_sb = [wp_pool.tile([128, C], BF16, tag="wp", name="wp") for _ in range(NP)]
        for p in range(NP):
            nc.sync.dma_start(wp_sb[p][:], wpT[p])
        wqk_sb = [[wqk_pool.tile([128, QC], F32R, tag="wqk", name="wqk")
                   for _ in range(CT)] for _ in range(2)]
        for half in range(2):
            for c in range(CT):
                nc.sync.dma_start(wqk_sb[half][c][:], wqkT[half, c])
        wv_sb = [wv_pool.tile([128, QC], F32R, tag="wv", name="wv") for _ in range(CT)]
        for c in range(CT):
            nc.sync.dma_start(wv_sb[c][:], wvT[c])

        qt = {}
        kt = {}
        vt = []
        yt = {}

        def body():
            qt.clear(); kt.clear(); vt.clear(); yt.clear()
            ctx2 = ExitStack()
            # PSUM: st 2x2 banks + o 2x1 + grp 2x1 = 8 banks
            st_pool = ctx2.enter_context(tc.tile_pool(name="st", bufs=2, space="PSUM"))
            o_pool = ctx2.enter_context(tc.tile_pool(name="o", bufs=2, space="PSUM"))
            grp_pool = ctx2.enter_context(tc.tile_pool(name="grp", bufs=2, space="PSUM"))

            class Sched:
                def __init__(self):
                    self.vpe = 0.0
                    self.vact = 0.0
                    self.vdve = 0.0
                    self.fillers = deque()

                def pe(self, ns):
                    self.vpe += ns

                def dve(self, ns, after=None):
                    t0 = max(self.vdve, self.vpe if after is None else after)
                    self.vdve = t0 + ns
                    return self.vdve

                def fill_to(self, t):
                    while self.vpe < t and self.fillers:
                        g = self.fillers[0]
                        try:
                            next(g)
                        except StopIteration:
                            self.fillers.popleft()

                def drain(self, g):
                    # force-complete one generator (removing it from fillers)
                    try:
                        self.fillers.remove(g)
                    except ValueError:
                        pass
                    for _ in g:
                        pass

            sch = Sched()

            def gen_proj(tb):
                """Phase A for block tb; yields after ~2 matmuls of work."""
                xs = [xs_pool.tile([128, TQ], F32R, tag="xs", name="xs")
                      for _ in range(CT)]
                for c in range(CT):
                    nc.sync.dma_start(xs[c][:], xT[c, tb])
                yield
                with nc.allow_low_precision(reason="bf16 attention operands"):
                    for half in range(2):
                        for jp in range(NP):
                            ps = grp_pool.tile([128, TQ], F32, tag="grp", name="mm")
                            for c in range(CT):
                                nc.tensor.matmul(
                                    ps[:], wqk_sb[half][c][:, 128 * jp:128 * (jp + 1)],
                                    xs[c][:], start=(c == 0), stop=(c == CT - 1))
                                if c % 2 == 1:
                                    sch.pe(2 * _mm(TQ))
                                    yield
                            pool = qt_pool if half == 0 else kt_pool
                            dst = pool.tile([128, TQ], BF16, tag="t", name="qk")
                            nc.vector.tensor_copy(dst[:], ps[:])
                            sch.dve(_dve(TQ, 120.0))
                            (qt if half == 0 else kt)[(jp, tb)] = dst
                            yield
                    for ti in range(TQ // 128):
                        ps = grp_pool.tile([128, QC], F32, tag="grp", name="mmv")
                        for c in range(CT):
                            nc.tensor.matmul(ps[:], xs[c][:, 128 * ti:128 * (ti + 1)],
                                             wv_sb[c][:], start=(c == 0), stop=(c == CT - 1))
                            if c % 2 == 1:
                                sch.pe(2 * _mm(QC))
                                yield
                        vtile = v_pool.tile([128, NH * (HD + 1)], BF16, tag="v", name="v")
                        v3 = vtile[:].rearrange("p (h d) -> p h d", d=HD + 1)
                        nc.vector.tensor_copy(v3[:, :, 0:HD],
                                              ps[:].rearrange("p (h d) -> p h d", d=HD))
                        nc.vector.tensor_copy(v3[:, :, HD], ones_sb[:])
                        sch.dve(_dve(QC, 120.0) + _dve(NH, 120.0))
                        assert len(vt) == tb * (TQ // 128) + ti
                        vt.append(vtile)
                        yield

            def gen_outproj(tb):
                """Phase C for block tb (filler work)."""
                for ot in range(NO):
                    ps = grp_pool.tile([128, TQ], F32, tag="grp", name="mmo")
                    for p in range(NP):
                        nc.tensor.matmul(ps[:], wp_sb[p][:, 128 * ot:128 * (ot + 1)],
                                         yt[(p, tb)][:], start=(p == 0), stop=(p == NP - 1))
                        if p % 2 == 1:
                            sch.pe(2 * _mm(TQ))
                            yield
                    osb = osb_pool.tile([128, TQ], F32, tag="osb", name="osb")
                    nc.vector.tensor_scalar_add(osb[:], ps[:], bias_sb[:, ot:ot + 1])
                    sch.dve(_dve(TQ))
                    nc.sync.dma_start(outT[ot, tb], osb[:])
                    yield

            o_free = [0.0, 0.0]  # virtual free time of the o ring slots

            def emit_attention(qi):
                tq0 = qi * TQ
                ntk = (tq0 + TQ) // 128
                st_free = [0.0, 0.0]
                with nc.allow_low_precision(reason="bf16 attention operands"):
                    for p in range(NP):
                        h0, h1 = 2 * p, 2 * p + 1
                        o0 = o_pool.tile([HD + 1, TQ], F32, tag="o", name="o0")
                        o1 = o_pool.tile([HD + 1, TQ], F32, tag="o", name="o1")
                        qtile = qt[(p, qi)]

                        pend = deque()  # (ready_ns, emit_av_fn, tki)

                        def emit_ready_avs(force_all=False, cap=2):
                            while pend and (force_all or len(pend) > cap
                                            or pend[0][0] <= sch.vpe):
                                ready, fn, _ = pend.popleft()
                                sch.fill_to(ready)
                                fn()

                        for tki in range(ntk):
                            tk0 = tki * 128
                            dlt = max(0, tk0 - tq0)
                            w = TQ - dlt
                            diag = tk0 >= tq0
                            ktile = kt[(p, tk0 // TQ)]
                            koff = tk0 % TQ
                            # st ring slot reuse: wait for exp(i-2) to clear
                            sch.fill_to(st_free[tki % 2])
                            st = st_pool.tile([128, 2 * TQ], F32, tag="st", name="st")
                            nc.tensor.matmul(st[:, 0:w], ktile[0:64, koff:koff + 128],
                                             qtile[0:64, dlt:TQ], start=True, stop=not diag)
                            nc.tensor.matmul(st[:, TQ:TQ + w], ktile[64:128, koff:koff + 128],
                                             qtile[64:128, dlt:TQ], start=True, stop=not diag)
                            sch.pe(_mm(w) + 4.0)  # row-groups (0,0)/(64,0) run concurrently
                            if diag:
                                # mask the 128-wide diagonal band: += -BIG triangle
                                nc.tensor.matmul(st[:, 0:128], eye_sb[:], negtri[:],
                                                 start=False, stop=True)
                                nc.tensor.matmul(st[:, TQ:TQ + 128], eye_sb[:], negtri[:],
                                                 start=False, stop=True)
                                sch.pe(2 * _mm(128))
                            pt = pt_pool.tile([128, 2 * TQ], BF16, tag="pt", name="pt")
                            st_v = st[:].rearrange("p (h q) -> p h q", q=TQ)[:, :, 0:w]
                            pt_v = pt[:, 0:2 * w].rearrange("p (h q) -> p h q", h=2)
                            nc.scalar.activation(pt_v, st_v, AF.Exp, scale=scale)
                            exp_end = max(sch.vact, sch.vpe) + _act(2 * w)
                            sch.vact = exp_end
                            st_free[tki % 2] = exp_end
                            ready = exp_end

                            def emit_av(pt_=pt, w_=w, dlt_=dlt, tki_=tki):
                                vtile = vt[tki_]
                                v3 = vtile[:].rearrange("p (h d) -> p h d", d=HD + 1)
                                nc.tensor.matmul(o0[:, dlt_:TQ], v3[:, h0, :], pt_[:, 0:w_],
                                                 start=(tki_ == 0), stop=(tki_ == ntk - 1))
                                nc.tensor.matmul(o1[:, dlt_:TQ], v3[:, h1, :], pt_[:, w_:2 * w_],
                                                 start=(tki_ == 0), stop=(tki_ == ntk - 1))
                                sch.pe(2 * _mm(w_))

                            if tki == 0:
                                ready = max(ready, o_free[0], o_free[1])
                            pend.append((ready, emit_av, tki))
                            emit_ready_avs()
                        emit_ready_avs(force_all=True)

                        # normalize: rc = 1/denoms; bc = broadcast via PE; y = o * bc
                        rcA = rc_pool.tile([1, TQ], F32R, tag="rca", name="rcA", bufs=2)
                        rcB = rc_pool.tile([1, TQ], F32R, tag="rcb", name="rcB", bufs=2)
                        nc.vector.reciprocal(rcA[:], o0[HD:HD + 1, :])
                        nc.vector.reciprocal(rcB[:], o1[HD:HD + 1, :])
                        rc_end = sch.dve(2 * _dve(TQ, 120.0))
                        sch.fill_to(rc_end)
                        bc0 = grp_pool.tile([HD, TQ], F32, tag="grp", name="bc0")
                        bc1 = grp_pool.tile([HD, TQ], F32, tag="grp", name="bc1")
                        nc.tensor.matmul(bc0[:], sel_sb[:], rcA[:], start=True, stop=True)
                        nc.tensor.matmul(bc1[:], sel_sb[:], rcB[:], start=True, stop=True)
                        sch.pe(2 * _mm(TQ))
                        bc_sb = rc_pool.tile([128, TQ], F32R, tag="bcs", name="bc_sb", bufs=2)
                        nc.vector.tensor_copy(bc_sb[0:64, :], bc0[:])
                        nc.vector.tensor_copy(bc_sb[64:128, :], bc1[:])
                        ytile = yt_pool.tile([128, TQ], BF16, tag="yt", name="y")
                        yt[(p, qi)] = ytile
                        nc.vector.tensor_mul(ytile[0:64, :], o0[0:HD, :], bc_sb[0:64, :])
                        nc.vector.tensor_mul(ytile[64:128, :], o1[0:HD, :], bc_sb[64:128, :])
                        mul_end = sch.dve(2 * _dve(TQ, 120.0) + 2 * _dve(TQ))
                        o_free[0] = o_free[1] = mul_end

            # ---- main emission ----
            proj_gens = [gen_proj(tb) for tb in range(NTB)]
            sch.drain(proj_gens[0])
            for qi in range(NTB):
                if qi + 1 < NTB:
                    sch.fillers.append(proj_gens[qi + 1])
                emit_attention(qi)
                if qi + 1 < NTB:
                    sch.drain(proj_gens[qi + 1])
                sch.fillers.append(gen_outproj(qi))
            while sch.fillers:
                g = sch.fillers.popleft()
                sch.drain(g)
            ctx2.close()

        if loop_iters == 1:
            body()
        else:
            with tc.For_i(0, loop_iters, 1):
                body()
    nc.finalize()
    return nc


def _tile2d(a, pr, pc):
    """[R, S] -> [R//pr, S//pc, pr, pc] contiguous tiles."""
    R, S = a.shape
    return np.ascontiguousarray(
        a.reshape(R // pr, pr, S // pc, pc).transpose(0, 2, 1, 3))


def shard_inputs(x, w_attn, w_proj, b_proj, TQ=512):
    """Returns in_maps for 8 cores: core c = (b=c//2, g=c%2)."""
    CT = C // 128
    NP = NH // 2
    bf = mybir.dt.np(BF16)
    wq, wk, wv = w_attn[0:C], w_attn[C:2 * C], w_attn[2 * C:3 * C]
    x = np.asarray(x)
    in_maps = []
    for core in range(8):
        b = core // 2
        g = core % 2
        rows = slice(g * QCOLS, (g + 1) * QCOLS)
        xTt = _tile2d(np.asarray(x[b]).T, 128, TQ)                 # [CT,NTB,128,TQ]
        wqkT_flat = np.concatenate([wq[rows], wk[rows]], 0).T      # [C, 2QC]
        wqkTt = np.ascontiguousarray(
            wqkT_flat.reshape(CT, 128, 2, QCOLS).transpose(2, 0, 1, 3))  # [2,CT,128,QC]
        wvTt = np.ascontiguousarray(wv[rows].T.reshape(CT, 128, QCOLS))
        wpTt = np.ascontiguousarray(
            w_proj[:, rows].T.reshape(NP, 128, C)).astype(bf)
        in_maps.append({
            "xT": xTt,
            "wqkT": wqkTt,
            "wvT": wvTt,
            "wpT": wpTt,
            "bias": (np.ascontiguousarray(b_proj.reshape(C // 128, 128).T)
                     if g == 0 else np.zeros((128, C // 128), np.float32)),
        })
    return in_maps


def unshard_output(outT_tiles_pair, TQ=512):
    """outT [NO,NTB,128,TQ] partials (2 cores) -> out [T, C]."""
    s = outT_tiles_pair[0] + outT_tiles_pair[1]
    NO, NTB = C // 128, T // TQ
    return s.transpose(0, 2, 1, 3).reshape(C, T).T


_NC_CACHE = {}


def kernel(x, w_attn, w_proj, b_proj):
    if "nc" not in _NC_CACHE:
        _NC_CACHE["nc"] = build()
    nc = _NC_CACHE["nc"]
    in_maps = shard_inputs(x, w_attn, w_proj, b_proj)
    res = run_bass_kernel_spmd(nc, in_maps, core_ids=list(range(8)))
    out = np.empty((B, T, C), np.float32)
    for b in range(B):
        out[b] = unshard_output([res.results[2 * b]["outT"],
                                 res.results[2 * b + 1]["outT"]])
    return out
